# revision 1
# baseline (speedup 1.0000x reference)
"""Bahdanau attention cell (location-sensitive) on 8 TRN2 NeuronCores.

Sharding: data-parallel over the batch dim (64 -> 8 batches/core); all
params (conv kernel, location dense, score v/b) are tiny and replicated.

Per-core device program (Bass/Tile):
  1. conv(prev_weights) as one block-diagonal matmul over an im2col matrix
     built by a single overlapping-window DMA from the host-padded
     alignments.  4 batches per matmul group -> f_all [4*32, T].
  2. main pass in [A-on-partitions, T-on-free] layout:
       PSUM[a, t]  = loc_w.T-chunk @ f[b]          (ploc, fp32r matmul)
                   + x[b, t, a]^T                  (PE transpose-accumulate)
       tanh via ACT with per-partition bias = (query[b]+score_b)[a-chunk]
       energy[b, t] += v[a-chunk] . tanh           (fp32r matmul with
                                                    zero-padded per-batch
                                                    stationary columns)
  3. masked softmax over t in [batch, T] layout on DVE/ACT, cumulative add,
     DMA out.
"""

import os
import sys

sys.path.insert(0, "/opt/trn_rl_repo")

import numpy as np

import concourse.bacc as bacc
import concourse.bass as bass
import concourse.tile as tile
from concourse import mybir
from concourse.bass_utils import run_bass_kernel_spmd

B, T, A, F, KW = 64, 2000, 256, 32, 31
NCORES = 8
BL = B // NCORES  # 8 batches per core
PAD = (KW - 1) // 2  # 15
TP = T + 2 * PAD  # 2030
F32 = mybir.dt.float32
F32R = mybir.dt.float32r

# t-ranges processed per outer iteration (PSUM z-tile free size = 1024 = 2 banks)
TR = [(0, 1024), (1024, T - 1024)]


def _halves(tsz):
    out = [(0, min(512, tsz))]
    if tsz > 512:
        out.append((512, tsz - 512))
    return out


def _r(ap):
    return ap.bitcast(F32R)


def build_program(stage="full"):
    nc = bacc.Bacc("TRN2", target_bir_lowering=False)

    wmem = nc.dram_tensor("wmem", [BL, T, A], F32, kind="ExternalInput")
    prevp = nc.dram_tensor("prevp", [BL, TP], F32, kind="ExternalInput")
    locw = nc.dram_tensor("locw", [2 * F, A], F32, kind="ExternalInput")
    bd = nc.dram_tensor("bd", [2 * KW, 64], F32, kind="ExternalInput")
    cb4 = nc.dram_tensor("cb4", [64, 1], F32, kind="ExternalInput")
    qbt = nc.dram_tensor("qbt", [A, BL], F32, kind="ExternalInput")
    vpad = nc.dram_tensor("vpad", [128, 128], F32, kind="ExternalInput")
    ident = nc.dram_tensor("ident", [128, 128], F32, kind="ExternalInput")
    maskd = nc.dram_tensor("mask", [BL, T], F32, kind="ExternalInput")
    out_w = nc.dram_tensor("out_w", [BL, T], F32, kind="ExternalOutput")
    out_nw = nc.dram_tensor("out_nw", [BL, T], F32, kind="ExternalOutput")

    with tile.TileContext(nc) as tc:
        with (
            tc.tile_pool(name="singles", bufs=1) as singles,
            tc.tile_pool(name="fpool", bufs=1) as fpool,
            tc.tile_pool(name="impool", bufs=2) as impool,
            tc.tile_pool(name="xpool", bufs=3) as xpool,
            tc.tile_pool(name="thpool", bufs=3) as thpool,
            tc.tile_pool(name="spool", bufs=1) as spool,
            tc.tile_pool(name="pz", bufs=2, space="PSUM") as pzpool,
            tc.tile_pool(name="pe", bufs=2, space="PSUM") as pepool,
        ):
            # ---- constants ----
            ident_sb = singles.tile([128, 128], F32, tag="ident")
            nc.sync.dma_start(out=ident_sb[:], in_=ident[:])
            locw_sb = singles.tile([2 * F, A], F32, tag="locw")
            nc.sync.dma_start(out=locw_sb[:], in_=locw[:])
            locw_r = singles.tile([2 * F, A], F32R, tag="locw_r")
            nc.vector.tensor_copy(out=locw_r[:], in_=locw_sb[:])
            bd_sb = singles.tile([2 * KW, 64], F32, tag="bd")
            nc.sync.dma_start(out=bd_sb[:], in_=bd[:])
            bd_r = singles.tile([2 * KW, 64], F32R, tag="bd_r")
            nc.vector.tensor_copy(out=bd_r[:], in_=bd_sb[:])
            cb_sb = singles.tile([64, 1], F32, tag="cb")
            nc.sync.dma_start(out=cb_sb[:], in_=cb4[:])
            vpad_sb = singles.tile([128, 128], F32, tag="vpad")
            nc.sync.dma_start(out=vpad_sb[:], in_=vpad[:])
            vpad_r = singles.tile([128, 128], F32R, tag="vpad_r")
            nc.vector.tensor_copy(out=vpad_r[:], in_=vpad_sb[:])
            qbt_sb = singles.tile([128, 2 * BL], F32, tag="qbt")
            nc.sync.dma_start(out=qbt_sb[:, 0:BL], in_=qbt[0:128, :])
            nc.sync.dma_start(out=qbt_sb[:, BL : 2 * BL], in_=qbt[128:256, :])
            mask_sb = singles.tile([BL, T], F32, tag="mask")
            nc.sync.dma_start(out=mask_sb[:], in_=maskd[:])
            prev_sb = singles.tile([BL, T], F32, tag="prev")
            nc.sync.dma_start(out=prev_sb[:], in_=prevp[:, PAD : PAD + T])
            # early DVE touches: make DVE observe these DMA sems up front so
            # late DVE consumers carry at most one fresh sync wait each
            # (TensorScalarPtr/TensorTensor have few HW wait slots).
            dve_warm = singles.tile([BL, 2], F32, tag="dve_warm")
            nc.vector.tensor_copy(out=dve_warm[:, 0:1], in_=mask_sb[:, 0:1])
            nc.vector.tensor_copy(out=dve_warm[:, 1:2], in_=prev_sb[:, 0:1])

            # ---- conv phase: f_all[g] [128, T], rows 32*bs + fc ----
            f_sb = []
            for g in range(4):
                fg = fpool.tile([64, T], F32R, tag=f"f{g}")
                f_sb.append(fg)
                im = impool.tile([2 * KW, T], F32, tag="im")
                base = prevp[2 * g : 2 * g + 2, :]
                src = bass.AP(
                    tensor=base.tensor,
                    offset=base.offset,
                    ap=[[TP, 2], [1, KW], [1, T]],
                )
                nc.sync.dma_start(out=im[:], in_=src)
                im_r = impool.tile([2 * KW, T], F32R, tag="im_r")
                nc.vector.tensor_copy(out=im_r[:], in_=im[:])
                for t0, tsz in TR:
                    pc = pzpool.tile([128, 1024], F32, tag="z")
                    for u0, un in _halves(tsz):
                        nc.tensor.matmul(
                            pc[0:64, u0 : u0 + un],
                            bd_r[:],
                            im_r[:, t0 + u0 : t0 + u0 + un],
                            start=True,
                            stop=True,
                        )
                    # evacuate with conv bias add (bias per partition = conv_b x2)
                    nc.scalar.activation(
                        out=fg[:, t0 : t0 + tsz],
                        in_=pc[0:64, 0:tsz],
                        func=mybir.ActivationFunctionType.Identity,
                        bias=cb_sb[:, 0:1],
                        scale=1.0,
                    )

            if stage == "conv":
                dbg = spool.tile([BL, T], F32, tag="dbg")
                nc.vector.tensor_copy(out=dbg[:], in_=f_sb[0][0:BL, :].bitcast(F32))
                nc.sync.dma_start(out=out_w[:], in_=dbg[:])
                nc.sync.dma_start(out=out_nw[:], in_=dbg[:])

            # ---- main pass ----
            energy_sb = spool.tile([BL, T], F32, tag="energy")
            tr_list = [] if stage == "conv" else TR
            for t0, tsz in tr_list:
                pe_e = pepool.tile([BL, 1024], F32, tag="e")
                halves = _halves(tsz)
                started = {u0: False for u0, _ in halves}
                njf, rem = tsz // 128, tsz % 128
                nj = njf + (1 if rem else 0)
                for b in range(BL):
                    g, bs = b // 2, b % 2
                    xb = xpool.tile([128, nj * A], F32, tag="x")
                    base = wmem[b, t0 : t0 + tsz, :]
                    if njf:
                        src = bass.AP(
                            tensor=base.tensor,
                            offset=base.offset,
                            ap=[[A, 128], [128 * A, njf], [1, A]],
                        )
                        nc.sync.dma_start(out=xb[:, 0 : njf * A], in_=src)
                    if rem:
                        nc.sync.dma_start(
                            out=xb[0:rem, njf * A : nj * A],
                            in_=wmem[b, t0 + njf * 128 : t0 + tsz, :],
                        )
                    for c in range(2):
                        pz = pzpool.tile([128, 1024], F32, tag="z")
                        for u0, un in halves:
                            nc.tensor.matmul(
                                pz[:, u0 : u0 + un],
                                locw_r[32 * bs : 32 * bs + 32, c * 128 : (c + 1) * 128],
                                f_sb[g][32 * bs : 32 * bs + 32, t0 + u0 : t0 + u0 + un],
                                start=True,
                                stop=False,
                            )
                        # stop only on the last matmul touching each PSUM bank
                        # (group tracking is bank-granular in the checker)
                        lastj_bank0 = min(nj, 4) - 1
                        lastj_bank1 = nj - 1
                        for j in range(nj):
                            pj = min(128, tsz - j * 128)
                            nc.tensor.matmul(
                                pz[:, j * 128 : j * 128 + pj],
                                xb[0:pj, j * A + c * 128 : j * A + c * 128 + 128],
                                ident_sb[0:pj, 0:pj],
                                start=False,
                                stop=(j == lastj_bank0 or j == lastj_bank1),
                                is_transpose=True,
                            )
                        th = thpool.tile([128, 1024], F32R, tag="th")
                        nc.scalar.activation(
                            out=th[:, 0:tsz],
                            in_=pz[:, 0:tsz],
                            func=mybir.ActivationFunctionType.Tanh,
                            bias=qbt_sb[:, BL * c + b : BL * c + b + 1],
                            scale=1.0,
                        )
                        blk = (c * BL + b) * 8
                        last = b == BL - 1 and c == 1
                        for u0, un in halves:
                            nc.tensor.matmul(
                                pe_e[:, u0 : u0 + un],
                                vpad_r[:, blk : blk + 8],
                                th[:, u0 : u0 + un],
                                start=not started[u0],
                                stop=last,
                            )
                            started[u0] = True
                nc.vector.tensor_copy(
                    out=energy_sb[:, t0 : t0 + tsz], in_=pe_e[:, 0:tsz]
                )

            if stage == "energy":
                nc.sync.dma_start(out=out_w[:], in_=energy_sb[:])
                nc.sync.dma_start(out=out_nw[:], in_=energy_sb[:])

            do_softmax = stage == "full"
            # ---- masked softmax + cumulative add ----
            scratch = spool.tile([BL, T], F32, tag="scratch")
            mx = spool.tile([BL, 1], F32, tag="mx")
            if not do_softmax:
                pass
            else:
                _softmax_tail(nc, spool, energy_sb, mask_sb, prev_sb, mx, scratch, out_w, out_nw)

    nc.finalize()
    return nc


def _softmax_tail(nc, spool, energy_sb, mask_sb, prev_sb, mx, scratch, out_w, out_nw):
            nc.vector.tensor_mul(scratch[:], energy_sb[:], mask_sb[:])
            nc.vector.tensor_reduce(
                out=mx[:], in_=scratch[:],
                axis=mybir.AxisListType.X, op=mybir.AluOpType.max,
            )
            negm = spool.tile([BL, 1], F32, tag="negm")
            nc.vector.tensor_scalar_mul(negm[:], mx[:], -1.0)
            e_sb = spool.tile([BL, T], F32, tag="e")
            nc.scalar.activation(
                out=e_sb[:],
                in_=energy_sb[:],
                func=mybir.ActivationFunctionType.Exp,
                bias=negm[:, 0:1],
                scale=1.0,
            )
            num_sb = spool.tile([BL, T], F32, tag="num")
            ssum = spool.tile([BL, 1], F32, tag="ssum")
            nc.vector.tensor_mul(num_sb[:], e_sb[:], mask_sb[:])
            nc.vector.tensor_reduce(
                out=ssum[:], in_=num_sb[:],
                axis=mybir.AxisListType.X, op=mybir.AluOpType.add,
            )
            rinv = spool.tile([BL, 1], F32, tag="rinv")
            nc.vector.reciprocal(rinv[:], ssum[:])
            ow_sb = spool.tile([BL, T], F32, tag="ow")
            nc.vector.tensor_scalar_mul(ow_sb[:], num_sb[:], rinv[:, 0:1])
            nw_sb = spool.tile([BL, T], F32, tag="nw")
            nc.vector.tensor_add(nw_sb[:], ow_sb[:], prev_sb[:])
            nc.sync.dma_start(out=out_w[:], in_=ow_sb[:])
            nc.sync.dma_start(out=out_nw[:], in_=nw_sb[:])


def make_in_maps(query, prev_weights, w_memory, memory_lengths, conv_w, conv_b,
                 loc_w, score_v, score_b):
    """Host-side prep (small params only) + batch sharding."""
    query = np.asarray(query, np.float32)
    prev_weights = np.asarray(prev_weights, np.float32)
    w_memory = np.ascontiguousarray(np.asarray(w_memory, np.float32))
    memory_lengths = np.asarray(memory_lengths)
    conv_w = np.asarray(conv_w, np.float32)
    conv_b = np.asarray(conv_b, np.float32)
    loc_w = np.asarray(loc_w, np.float32)
    score_v = np.asarray(score_v, np.float32)
    score_b = np.asarray(score_b, np.float32)

    # block-diagonal conv kernel: bd[31*bs + k, 32*bs + fc] = conv_w[k, 0, fc]
    bd = np.zeros((2 * KW, 64), np.float32)
    for bs in range(2):
        bd[KW * bs : KW * (bs + 1), 32 * bs : 32 * (bs + 1)] = conv_w[:, 0, :]
    cb4 = np.tile(conv_b, 2).reshape(64, 1).astype(np.float32)
    # zero-padded per-batch stationary columns for the score_v dot
    vp = np.zeros((128, 128), np.float32)
    for c in range(2):
        for b in range(BL):
            vp[:, (c * BL + b) * 8 + b] = score_v[c * 128 : (c + 1) * 128]
    ident = np.eye(128, dtype=np.float32)
    qb = query + score_b[None, :]  # [B, A]
    prevp_full = np.pad(prev_weights, ((0, 0), (PAD, PAD)))

    in_maps = []
    for i in range(NCORES):
        s = slice(i * BL, (i + 1) * BL)
        in_maps.append(
            {
                "wmem": np.ascontiguousarray(w_memory[s]),
                "prevp": np.ascontiguousarray(prevp_full[s]),
                "locw": np.tile(loc_w, (2, 1)),
                "bd": bd,
                "cb4": cb4,
                "qbt": np.ascontiguousarray(qb[s].T),
                "vpad": vp,
                "ident": ident,
                "mask": (np.arange(T)[None, :] < np.asarray(memory_lengths)[s][:, None]).astype(np.float32),
            }
        )
    return in_maps


_NC_CACHE = {}


def _get_nc():
    if "nc" not in _NC_CACHE:
        _NC_CACHE["nc"] = build_program()
    return _NC_CACHE["nc"]


def run(inputs, trace=False, tmpdir=None):
    """Run on 8 NeuronCores; returns ((output, new_weights), BassKernelResults)."""
    nc = _get_nc()
    in_maps = make_in_maps(**inputs)
    res = run_bass_kernel_spmd(
        nc, in_maps, core_ids=list(range(NCORES)), trace=trace, tmpdir=tmpdir
    )
    output = np.concatenate([res.results[i]["out_w"] for i in range(NCORES)], axis=0)
    new_w = np.concatenate([res.results[i]["out_nw"] for i in range(NCORES)], axis=0)
    return (output.astype(np.float32), new_w.astype(np.float32)), res


def kernel(**inputs):
    (output, new_w), _ = run(inputs, trace=False)
    return output, new_w

